# revision 16
# baseline (speedup 1.0000x reference)
"""Trainium2 Bass kernel for nn_AudioSNN: 2-layer spiking NN (snntorch Leaky).

Reference semantics per timestep t (over T=200 steps):
    cur1 = x_t @ w1.T + b1                      # [B, 128]
    m1   = 0.9*m1 + cur1 - (m1_prev > 1)        # reset-by-subtract
    spk1 = (m1 > 1)
    cur2 = spk1 @ w2.T + b2                     # [B, 5]
    m2   = 0.9*m2 + cur2 - (m2_prev > 1)
    out[t] = spk2 = (m2 > 1)

Strategy (pure data-parallel over batch, 8 cores x 1024 batch rows):
  - Transposed layout: states kept as [feature, batch]; H=128 on SBUF
    partitions, batch on the free dim.
  - The only loop-carried dependency on the device is the m1 membrane
    update (custom fused DVE op, one [128,1024] op per step).  The
    sign activation (ACT), layer-2 matmul (PE, lagged 2 steps), PSUM
    evacuation (Pool) and DMAs are all feed-forward and pipeline
    behind it.  Engine queues never round-trip within a step:
      PE:   mm1(t) mm2(t-2) | mm1(t+1) mm2(t-1) | ...
      DVE:  upd1(t)         | upd1(t+1)         | ...
      ACT:  sign(t)         | sign(t+1)         | ...
      Pool: copy p2(t-2)->SBUF, issue out-DMA    | ...
  - fp16 hi/lo splits keep matmul accuracy at fp32-ish level:
    mm1 = wh@xh + wh@xl + wl@xh as ONE K=120-stacked pass;
    mm2 = w2h@sg + w2l@sg (2 passes, 4 column-tile groups).
  - The tiny layer-2 membrane scan (B x 5 per step) runs on the host:
    the device streams raw cur2 (PSUM -> Pool copy -> DMA out), and
    the host applies the m2 recurrence + thresholds.
"""

import numpy as np

import concourse.bacc as bacc
import concourse.mybir as mybir
import concourse.tile as tile
import concourse.dve_ops as dve_ops
from concourse.dve_ops import DveOp
from concourse.dve_spec import Spec, Src0, Src1, C0, C1, C2, lower as dve_lower
from concourse.dve_uop import DveOpSpec
from concourse.bass_utils import run_bass_kernel_spmd

F32 = mybir.dt.float32
F16 = mybir.dt.float16

B, T, F, H, C = 8192, 200, 40, 128, 5
NCORES = 8
BL = B // NCORES          # 1024 batch rows per core
BH = BL // 2              # 512 per mm1 half (PSUM bank limit)
BETA, THR = 0.9, 1.0
NG = 4                    # col-tile groups for layer 2
BG = BL // NG             # 256 batch rows per col group
XB = 8                    # timesteps per x DMA batch
LAG = 4                   # mm2 runs LAG steps behind mm1


# --------------------------------------------------------------------------
# Custom DVE op: fused SNN membrane update
# --------------------------------------------------------------------------

def _snn_ref(in0, in1, s0, s1, imm2):
    out = (
        in0.astype(np.float32) * imm2
        - (in0 > s1).astype(np.float32)
        + in1.astype(np.float32)
        + s0
    )
    return out.astype(np.float32)


def _register_snn_op() -> DveOp:
    """out = in0*imm2 - (in0 > s1) + in1 + s0"""
    name = "SNN_MEMBRANE_STEP"
    for op in dve_ops.OPS:
        if op.name == name:
            return op
    body = Src0 * C2 - (Src0 > C1) + Src1 + C0
    spec = Spec(body=body, reference=_snn_ref)
    shas = {}
    for ver in ("v3", "v4"):
        uops = dve_lower(spec, ver=ver)
        shas[ver] = DveOpSpec(name=name, opcode=0, uops=uops, rd1_en=True).sha(ver)
    op = DveOp(name, spec, subdim=False, uops_sha=shas)
    dve_ops.OPS.append(op)
    dve_ops._SUB_OPCODE_FOR_NAME[op.name] = (
        dve_ops._CUSTOM_DVE_ROW_BASE + len(dve_ops.OPS) - 1
    )
    dve_ops.CUSTOM_DVE_SPECS[op.name] = spec
    return op


SNN_OP = _register_snn_op()


# --------------------------------------------------------------------------
# Bass module
# --------------------------------------------------------------------------

def build_module(t_steps: int = T, probe: str = ""):
    import os
    probe = probe or os.environ.get("KPROBE", "")
    assert t_steps % XB == 0
    tb = t_steps // XB
    nc = bacc.Bacc("TRN2", target_bir_lowering=False, debug=False)

    # x packed for the K-stacked 3-pass mm1: rows 0-39 = xh, rows 40-79
    # = xl, rows 80-119 = xh again (pairs with [wh; wh; wl] on the weight
    # side).  XB steps side by side in the free dim.
    XW = XB * BL
    xq = nc.dram_tensor("xq", [tb, 120, XW], F16, kind="ExternalInput").ap()
    # w1 fp16 triple-K stack [wh; wh; wl]
    w1trip = nc.dram_tensor("w1trip", [120, H], F16, kind="ExternalInput").ap()
    # w2 fp16 pair (padded to 32 cols)
    w2qh = nc.dram_tensor("w2qh", [H, 32], F16, kind="ExternalInput").ap()
    w2ql = nc.dram_tensor("w2ql", [H, 32], F16, kind="ExternalInput").ap()
    bias1 = nc.dram_tensor("bias1", [H, 1], F32, kind="ExternalInput").ap()
    # out[t, 32g+c, j] = cur2-ish for class c, batch b = g*BG + j at step t
    out = nc.dram_tensor("out", [t_steps, 128, BG], F32, kind="ExternalOutput").ap()

    with tile.TileContext(nc) as tc:
        with (
            tc.tile_pool(name="const", bufs=1) as cpool,
            tc.tile_pool(name="state", bufs=1) as spool,
            tc.tile_pool(name="xin", bufs=3) as xpool,
            tc.tile_pool(name="sgn", bufs=6) as gpool,
            tc.tile_pool(name="stage", bufs=4) as stpool,
            tc.tile_pool(name="ps1", bufs=3, space="PSUM") as p1pool,
            tc.tile_pool(name="ps2", bufs=2, space="PSUM") as p2pool,  # [128, 2*BG] = 1 bank each
        ):
            w1t_s = cpool.tile([120, H], F16)
            w2qh_s = cpool.tile([H, 32], F16)
            w2ql_s = cpool.tile([H, 32], F16)
            b1_s = cpool.tile([H, 1], F32)
            nc.sync.dma_start(w1t_s[:], w1trip[:])
            nc.sync.dma_start(w2qh_s[:], w2qh[:])
            nc.sync.dma_start(w2ql_s[:], w2ql[:])
            nc.sync.dma_start(b1_s[:], bias1[:])

            m1_bufs = [
                spool.tile([H, BL], F32, tag=f"m1{i}", name=f"m1{i}")
                for i in range(4)
            ]
            if probe == "no_dve":
                for mb in m1_bufs:
                    nc.gpsimd.memset(mb[:], 0.0)
            else:
                nc.gpsimd.memset(m1_bufs[3][:], 0.0)

            sg_pairs = {}
            x_tiles = {}
            statics = {}
            if probe == "no_act":
                sgst = spool.tile([H, 2 * BL], F16, tag="sgst", name="sgst")
                nc.gpsimd.memset(sgst[:], 1.0)
                statics["sg"] = sgst

            def l2_pair(p):
                """Layer-2 for step pair (2p, 2p+1): one 8-matmul batch
                (both steps' columns per group, weight-adjacent ordering),
                PSUM bank [128, 2*BG], evacuation alternating ACT/DVE,
                one combined out-DMA issued from Pool."""
                if probe == "no_l2":
                    sg_pairs.pop(p, None)
                    return
                sg = sg_pairs.pop(p, None) or statics.get("sg")
                p2 = p2pool.tile([128, 2 * BG], F32, tag="p2", name="p2")
                for wq, st_, sp_ in ((w2qh_s, True, False), (w2ql_s, False, True)):
                    for g in range(NG):
                        gs = sg[:, 2 * BG * g : 2 * BG * (g + 1)]
                        nc.tensor.matmul(
                            p2[32 * g : 32 * (g + 1), :], wq[:], gs,
                            start=st_, stop=sp_, tile_position=(0, 32 * g),
                        )
                if probe == "no_evac":
                    return
                st = stpool.tile([128, 2 * BG], F32, tag="st")
                if p % 2 == 0:
                    nc.scalar.copy(st[:], p2[:])
                else:
                    nc.vector.tensor_copy(st[:], p2[:])
                if probe == "no_outdma":
                    return
                tau = 2 * p
                nc.gpsimd.dma_start(
                    out[tau : tau + 2].rearrange("t p j -> p t j"),
                    st[:].rearrange("p (t j) -> p t j", t=2),
                )

            m1_prev = m1_bufs[3]
            for t in range(t_steps):
                k, s = divmod(t, XB)

                if s == 0:
                    if probe == "no_xdma":
                        if "xt" not in statics:
                            xst = spool.tile([120, XW], F16, tag="xst", name="xst")
                            nc.sync.dma_start(xst[:], xq[0])
                            statics["xt"] = xst
                        xt = statics["xt"]
                    else:
                        if k == 0:
                            for kk in (0, min(1, tb - 1)):
                                xt = xpool.tile([120, XW], F16, tag="x", name="xt")
                                nc.sync.dma_start(xt[:], xq[kk])
                                x_tiles[kk] = xt
                        elif k + 1 < tb:
                            xt = xpool.tile([120, XW], F16, tag="x", name="xt")
                            nc.sync.dma_start(xt[:], xq[k + 1])
                            x_tiles[k + 1] = xt
                        xt = x_tiles.pop(k)

                # mm1: cur1 = w1 @ x via one K=120 stacked pass
                # ([wh; wh; wl] . [xh; xl; xh]), split in two N=512 halves
                if probe == "no_mm1":
                    if "p1" not in statics:
                        p1st = p1pool.tile([H, BL], F32, tag="p1")
                        for half in (0, BH):
                            nc.tensor.matmul(
                                p1st[:, half : half + BH], w1t_s[:],
                                xt[:, half : half + BH], start=True, stop=True,
                            )
                        statics["p1"] = p1st
                    p1 = statics["p1"]
                else:
                    p1 = p1pool.tile([H, BL], F32, tag="p1")
                    for half in (0, BH):
                        nc.tensor.matmul(
                            p1[:, half : half + BH],
                            w1t_s[:],
                            xt[:, s * BL + half : s * BL + half + BH],
                            start=True, stop=True,
                        )

                # m1 = beta*m1 - (m1 > 1) + cur1 + b1
                m1 = m1_bufs[t % 4]
                if probe != "no_dve":
                    nc.vector._custom_dve(
                        SNN_OP, out=m1[:], in0=m1_prev[:], in1=p1[:],
                        s0=b1_s[:, 0:1], s1=THR, imm2=BETA,
                    )
                m1_prev = m1

                # sg = sign(1 - m1)  (= -sign(m1-1); spk1 = (1 - sg)/2),
                # written into the step-pair tile for batched mm2.  Layout
                # is group-major: col g*2*BG + sh*BG + j so each group's
                # two steps are contiguous for the mm2 rhs.
                p, sh = divmod(t, 2)
                if probe != "no_act":
                    if sh == 0:
                        sgp = gpool.tile([H, 2 * BL], F16, tag="sg", name="sgp")
                        sg_pairs[p] = sgp
                    sgp = sg_pairs[p]
                    dstv = sgp[:].rearrange("h (g s j) -> h g s j", g=NG, s=2)
                    nc.scalar.activation(
                        dstv[:, :, sh, :], m1[:].rearrange("h (g j) -> h g j", g=NG),
                        mybir.ActivationFunctionType.Sign,
                        bias=1.0, scale=-1.0,
                    )

                # layer-2 for the pair finished LAG steps ago (keeps PE fed:
                # mm1(t+1) is queued before mm2's pair needs its sg)
                if t >= 2 * LAG - 1 and t % 2 == 1:
                    l2_pair((t - (2 * LAG - 1)) // 2)

            for p in sorted(list(sg_pairs)):
                l2_pair(p)

    nc.compile()
    return nc


_MODULE_CACHE: dict = {}


def _get_module(t_steps: int = T):
    if t_steps not in _MODULE_CACHE:
        _MODULE_CACHE[t_steps] = build_module(t_steps)
    return _MODULE_CACHE[t_steps]


# --------------------------------------------------------------------------
# Host-side sharding / gather
# --------------------------------------------------------------------------

def _fp16_pair(a):
    hi = a.astype(np.float16)
    lo = (a - hi.astype(np.float32)).astype(np.float16)
    return hi, lo


def make_in_maps(x, w1, b1, w2, b2, t_steps: int = T):
    x = np.asarray(x, dtype=np.float32)
    w1 = np.asarray(w1, dtype=np.float32)
    b1 = np.asarray(b1, dtype=np.float32)
    w2 = np.asarray(w2, dtype=np.float32)
    tb = t_steps // XB

    w1h, w1l = _fp16_pair(w1.T)                           # [F, H] each
    w1trip = np.zeros((120, H), np.float16)
    w1trip[0:F] = w1h
    w1trip[F : 2 * F] = w1h
    w1trip[2 * F : 3 * F] = w1l

    w2nh, w2nl = _fp16_pair((-0.5 * w2).T)                # [H, C]
    w2qh = np.zeros((H, 32), np.float16)
    w2ql = np.zeros((H, 32), np.float16)
    w2qh[:, :C] = w2nh
    w2ql[:, :C] = w2nl

    bias1 = np.ascontiguousarray(b1[:, None])

    in_maps = []
    for c in range(NCORES):
        xc = x[c * BL : (c + 1) * BL, :t_steps, :]        # [BL, t, F]
        xt_ = xc.transpose(1, 2, 0)                       # [t, F, BL]
        xh16, xl16 = _fp16_pair(xt_)
        trip = np.concatenate([xh16, xl16, xh16], axis=1)  # [t, 120, BL]
        xqc = (
            trip.reshape(tb, XB, 120, BL)
            .transpose(0, 2, 1, 3)
            .reshape(tb, 120, XB * BL)
        )
        in_maps.append(
            {
                "xq": np.ascontiguousarray(xqc),
                "w1trip": w1trip,
                "w2qh": w2qh,
                "w2ql": w2ql,
                "bias1": bias1,
            }
        )
    return in_maps


def postprocess(results, w2, b2, t_steps: int = T):
    """results: per-core dicts with 'out' [t, 128, BG] = q = -0.5*w2f16 @ sg.

    cur2 = q - sum_h(w2q_eff[h, c]) + b2;  then run the m2 scan + threshold.
    """
    w2 = np.asarray(w2, dtype=np.float32)
    b2 = np.asarray(b2, dtype=np.float32)
    w2nh, w2nl = _fp16_pair((-0.5 * w2).T)                # [H, C] fp16
    w_eff = w2nh.astype(np.float32) + w2nl.astype(np.float32)
    corr = (-w_eff.sum(axis=0) + b2).astype(np.float32)   # [C]

    # q: [t, NCORES, NG, 32, BG] -> cur2 [t, B, C]
    qs = np.stack([results[c]["out"] for c in range(NCORES)], axis=1)
    q = qs.reshape(t_steps, NCORES, 128, BG)
    q = q.reshape(t_steps, NCORES, NG, 32, BG)[:, :, :, :C, :]
    cur2 = q.transpose(0, 1, 2, 4, 3).reshape(t_steps, B, C) + corr

    m2 = np.zeros((B, C), np.float32)
    spk2 = np.empty((t_steps, B, C), np.float32)
    for t in range(t_steps):
        r = (m2 > THR).astype(np.float32)
        m2 = BETA * m2 + cur2[t] - r * THR
        spk2[t] = m2 > THR
    return spk2


def kernel(x, w1, b1, w2, b2):
    nc = _get_module(T)
    in_maps = make_in_maps(x, w1, b1, w2, b2, T)
    res = run_bass_kernel_spmd(nc, in_maps, core_ids=list(range(NCORES)))
    return postprocess(res.results, w2, b2, T)
